# revision 10
# baseline (speedup 1.0000x reference)
"""Trainium2 Bass kernel for nn_EruSelfAttentionModel.

Model (reference):
    e = LayerNorm(emb_table[x]) * gamma + beta                      [B,S,E]
    q,k = per-head projections of e; scores = q @ k^T / sqrt(E)     [B,H,S,S]
    norm = minmax-normalized scores row; sel = max(norm) == 1.0 exactly
    weights = (1-sel)*softmax(norm) + sel*sigmoid(20*norm-10)       -> sigmoid only
    ov = weights @ (W_value @ e); out = sigmoid(fc(concat heads))   [B,S]
    return out[:, -1]                                               [B]

Only the last query position survives, and sel == 1.0 exactly in fp32
(it is (smax-smin)/(smax-smin)), so the softmax branch is multiplied by
exactly zero.  By linearity the value projection and fc fold into a single
vector per head g[h] = fc_w[h*E:(h+1)*E] @ W_value[h], and the score row
folds into qk[b,h] = (W_query[h] @ e_last) @ W_key[h].  Everything the
device must compute per token t is then two groups of dot products
    scores[b,h,t] = LN(e_t) . qk[b,h]/sqrt(E),   p[b,h,t] = LN(e_t) . g[h]
plus per-token LayerNorm statistics.  LN folds through the dot product:
    LN(e) . v = r * (e . (gamma*v) - mu * sum(gamma*v)) + beta . v
so the device computes raw = e_raw^T-block matmuls against a [E, 17]
matrix M = [gamma*qk/SCALE | gamma*g | ones] plus sum(e^2) (via an
on-chip square and a ones-column matmul).  The tiny per-(b,h) epilogue
(min/max over t, sigmoid weights, final weighted sum, output sigmoid)
is O(B*H*S) and runs on host after gathering 36 KB per core.

Sharding: the B*S = 4096 token rows are split into 8 contiguous blocks of
512, one per NeuronCore (cores 0-3 -> batch 0, cores 4-7 -> batch 1).
"""

import math

import numpy as np

B, S, E, A, H = 2, 2048, 512, 64, 8
NCORES = 8
ROWS = B * S // NCORES          # 512 token rows per core
NCH = E // 128                  # 4 contraction chunks of 128
J = 2 * H + 1                   # 8 score cols + 8 p cols + ones col
SCALE = math.sqrt(E)
EPS = 1e-5

_NC_CACHE = None


def build_nc():
    """Build the per-core Bass program (same program on all 8 cores)."""
    import concourse.bacc as bacc
    import concourse.tile as tile
    from concourse import mybir

    f32 = mybir.dt.float32
    bf16 = mybir.dt.bfloat16
    nc = bacc.Bacc("TRN2", target_bir_lowering=False)

    # ep: per-core fused input, [128, NCH, ROWS + J] bf16:
    #     ep[p, n, :ROWS] = e^T rows (feature d = n*128+p, token t on free dim)
    #     ep[p, n, ROWS:] = M[d, :]  (fold-matrix columns for this core's batch)
    ep = nc.dram_tensor("ep", [128, NCH, ROWS + J], bf16, kind="ExternalInput")
    # out rows 0..15: raw dot products e_t . M[:, j]; row 16: sum_d e; row 17: sum_d e^2
    out = nc.dram_tensor("out", [J + 1, ROWS], f32, kind="ExternalOutput")

    with tile.TileContext(nc) as tc:
        with (
            tc.tile_pool(name="sb", bufs=1) as sb,
            tc.tile_pool(name="ps", bufs=1, space="PSUM") as ps,
        ):
            ep_t = sb.tile([128, NCH, ROWS + J], bf16)
            esq_t = sb.tile([128, NCH, ROWS], bf16)
            out_t = sb.tile([J, ROWS], f32)
            q_t = sb.tile([1, ROWS], f32)
            po = ps.tile([J, ROWS], f32)
            pq = ps.tile([1, ROWS], f32)

            nc.sync.dma_start(out=ep_t[:, :, :], in_=ep[:, :, :])
            for n in range(NCH):
                nc.vector.tensor_mul(
                    esq_t[:, n, :], ep_t[:, n, :ROWS], ep_t[:, n, :ROWS]
                )
            # raw[j, t] = sum_d M[d, j] * e[d, t]  (j = 16 is the ones column -> sum_d e)
            for n in range(NCH):
                nc.tensor.matmul(
                    po[:, :], ep_t[:, n, ROWS:], ep_t[:, n, :ROWS],
                    start=(n == 0), stop=(n == NCH - 1),
                )
            # sum_d e^2 via the ones column against the squared tile
            for n in range(NCH):
                nc.tensor.matmul(
                    pq[:, :], ep_t[:, n, ROWS + J - 1:ROWS + J], esq_t[:, n, :],
                    start=(n == 0), stop=(n == NCH - 1),
                )
            nc.vector.tensor_copy(out_t[:, :], po[:, :])
            nc.scalar.copy(q_t[:, :], pq[:, :])
            nc.sync.dma_start(out=out[0:J, :], in_=out_t[:, :])
            nc.sync.dma_start(out=out[J:J + 1, :], in_=q_t[:, :])
    nc.finalize()
    return nc


def _get_nc():
    global _NC_CACHE
    if _NC_CACHE is None:
        _NC_CACHE = build_nc()
    return _NC_CACHE


def _sigmoid64(z):
    return 1.0 / (1.0 + np.exp(-z.astype(np.float64)))


def host_prep(x, emb, gamma, beta, Wq, Wk, Wv, fc_w):
    """Fold weights and shard inputs -> (in_maps, s_vec, c_vec)."""
    f32 = np.float32
    # q at the last position only (full LN of 2 rows, on host)
    er_last = emb[x[:, -1]]                                   # [B,E]
    mu = er_last.mean(-1, keepdims=True)
    var = ((er_last - mu) ** 2).mean(-1, keepdims=True)
    e_last = ((er_last - mu) / np.sqrt(var + EPS)) * gamma + beta
    q = np.einsum("had,bd->bha", Wq, e_last).astype(f32)      # [B,H,A]
    qk = np.einsum("bha,had->bhd", q, Wk).astype(f32)         # [B,H,E]
    g = np.einsum("hv,hvd->hd", fc_w[0].reshape(H, E), Wv).astype(f32)  # [H,E]

    Mb = np.empty((B, E, J), f32)
    Mb[:, :, :H] = (qk * gamma / SCALE).transpose(0, 2, 1)
    Mb[:, :, H:2 * H] = (g * gamma).T[None]
    Mb[:, :, 2 * H] = 1.0
    s_vec = Mb[:, :, :2 * H].sum(axis=1)                      # [B,16] col sums
    c_vec = np.concatenate(
        [(qk * beta).sum(-1) / SCALE, np.broadcast_to((g * beta).sum(-1), (B, H))],
        axis=1,
    ).astype(f32)                                             # [B,16]

    import ml_dtypes
    bf16 = ml_dtypes.bfloat16
    # device layout [partition p, chunk n, ROWS+J]: e^T block then M block
    mm_dev = [
        Mb[b].reshape(NCH, 128, J).transpose(1, 0, 2).astype(bf16) for b in range(B)
    ]
    er = emb[x.reshape(-1)]                                   # [B*S, E] gathered rows
    in_maps = []
    for c in range(NCORES):
        blk = er[c * ROWS:(c + 1) * ROWS]                     # [ROWS, E]
        b = (c * ROWS) // S
        ep = np.empty((128, NCH, ROWS + J), dtype=bf16)
        ep[:, :, :ROWS] = blk.T.astype(bf16).reshape(NCH, 128, ROWS).transpose(1, 0, 2)
        ep[:, :, ROWS:] = mm_dev[b]
        in_maps.append({"ep": ep})
    return in_maps, s_vec, c_vec


def host_epilogue(outs, s_vec, c_vec, fc_b):
    """outs: [NCORES, J+1, ROWS] device results -> final [B] output."""
    f32 = np.float32
    raw = outs[:, :2 * H, :]                                  # [8,16,512]
    mu = outs[:, 2 * H, :] / E                                # [8,512]
    ex2 = outs[:, 2 * H + 1, :] / E
    var = ex2 - mu * mu
    r = (1.0 / np.sqrt(var + f32(EPS))).astype(f32)

    bidx = (np.arange(NCORES) * ROWS) // S
    cols = (r[:, None, :] * (raw - mu[:, None, :] * s_vec[bidx][:, :, None])
            + c_vec[bidx][:, :, None])                        # [8,16,512]
    cols = cols.reshape(B, 4, 2 * H, ROWS).transpose(0, 2, 1, 3).reshape(B, 2 * H, S)
    scores = cols[:, :H, :]
    p = cols[:, H:, :]

    smax = scores.max(-1, keepdims=True)
    smin = scores.min(-1, keepdims=True)
    norm = (scores - smin) / (smax - smin)
    w = _sigmoid64(norm * f32(20.0) - f32(10.0))
    logit = (w * p.astype(np.float64)).sum((1, 2)) + np.float64(fc_b[0])
    return _sigmoid64(np.asarray(logit)).astype(f32)          # [B]


def kernel(x, emb_table, ln_gamma, ln_beta, W_query, W_key, W_value, fc_w, fc_b):
    f32 = np.float32
    x = np.asarray(x)
    emb = np.asarray(emb_table, dtype=f32)
    gamma = np.asarray(ln_gamma, dtype=f32)
    beta = np.asarray(ln_beta, dtype=f32)
    Wq = np.asarray(W_query, dtype=f32)
    Wk = np.asarray(W_key, dtype=f32)
    Wv = np.asarray(W_value, dtype=f32)
    fcw = np.asarray(fc_w, dtype=f32)
    fcb = np.asarray(fc_b, dtype=f32)

    in_maps, s_vec, c_vec = host_prep(x, emb, gamma, beta, Wq, Wk, Wv, fcw)

    from concourse.bass_utils import run_bass_kernel_spmd
    res = run_bass_kernel_spmd(_get_nc(), in_maps, core_ids=list(range(NCORES)))
    outs = np.stack([r["out"] for r in res.results])          # [8, J+1, ROWS]

    return host_epilogue(outs, s_vec, c_vec, fcb)


# revision 13
# speedup vs baseline: 1.0609x; 1.0609x over previous
"""Trainium2 Bass kernel for nn_EruSelfAttentionModel.

Model (reference):
    e = LayerNorm(emb_table[x]) * gamma + beta                      [B,S,E]
    q,k = per-head projections of e; scores = q @ k^T / sqrt(E)     [B,H,S,S]
    norm = minmax-normalized scores row; sel = max(norm) == 1.0 exactly
    weights = (1-sel)*softmax(norm) + sel*sigmoid(20*norm-10)       -> sigmoid only
    ov = weights @ (W_value @ e); out = sigmoid(fc(concat heads))   [B,S]
    return out[:, -1]                                               [B]

Only the last query position survives, and sel == 1.0 exactly in fp32
(it is (smax-smin)/(smax-smin)), so the softmax branch is multiplied by
exactly zero.  By linearity the value projection and fc fold into a single
vector per head g[h] = fc_w[h*E:(h+1)*E] @ W_value[h], and the score row
folds into qk[b,h] = (W_query[h] @ e_last) @ W_key[h].  Everything the
device must compute per token t is then two groups of dot products
    scores[b,h,t] = LN(e_t) . qk[b,h]/sqrt(E),   p[b,h,t] = LN(e_t) . g[h]
plus per-token LayerNorm statistics.  LN folds through the dot product:
    LN(e) . v = r * (e . (gamma*v) - mu * sum(gamma*v)) + beta . v
so the device computes raw = e_raw^T-block matmuls against a [E, 17]
matrix M = [gamma*qk/SCALE | gamma*g | ones] plus sum(e^2) (via an
on-chip square and a ones-column matmul).  The tiny per-(b,h) epilogue
(min/max over t, sigmoid weights, final weighted sum, output sigmoid)
is O(B*H*S) and runs on host after gathering 36 KB per core.

Sharding: the B*S = 4096 token rows are split into 8 contiguous blocks of
512, one per NeuronCore (cores 0-3 -> batch 0, cores 4-7 -> batch 1).
"""

import math

import numpy as np

B, S, E, A, H = 2, 2048, 512, 64, 8
NCORES = 8
ROWS = B * S // NCORES          # 512 token rows per core
NCH = E // 128                  # 4 contraction chunks of 128
JL = 2 * H + 2                  # 16 data cols + sum_e col + sum_e^2 col
CW = ROWS + 2 * JL              # per-chunk free width: e block | M block | Z block
SCALE = math.sqrt(E)
EPS = 1e-5

_NC_CACHE = None


def build_nc():
    """Build the per-core Bass program (same program on all 8 cores)."""
    import concourse.bacc as bacc
    import concourse.tile as tile
    from concourse import mybir

    f32 = mybir.dt.float32
    bf16 = mybir.dt.bfloat16
    nc = bacc.Bacc("TRN2", target_bir_lowering=False, enable_partition_id=False)

    # ep: per-core fused input, [128, NCH, CW] bf16, per chunk n:
    #     ep[p, n, :ROWS]            = e^T (feature d = n*128+p, token t on free)
    #     ep[p, n, ROWS:ROWS+JL]     = M[d, :] fold matrix (col 16 = ones, col 17 = 0)
    #     ep[p, n, ROWS+JL:]         = Z[d, :] (col 17 = ones, rest 0)
    ep = nc.dram_tensor("ep", [128, NCH, CW], bf16, kind="ExternalInput")
    # out rows 0..15: raw dot products e_t . M[:, j]; row 16: sum_d e; row 17: sum_d e^2
    out = nc.dram_tensor("out", [JL, ROWS], f32, kind="ExternalOutput")

    with tile.TileContext(nc) as tc:
        with (
            tc.tile_pool(name="sb", bufs=1) as sb,
            tc.tile_pool(name="ps", bufs=1, space="PSUM") as ps,
        ):
            ep_t = sb.tile([128, NCH, CW], bf16)
            esq_t = sb.tile([128, NCH, ROWS], bf16)
            out_t = sb.tile([JL, ROWS], f32)
            po = ps.tile([JL, ROWS], f32)

            # chunked loads, issue split across the two HWDGE queues
            for n in range(NCH):
                eng = nc.sync if n % 2 == 0 else nc.scalar
                eng.dma_start(out=ep_t[:, n, :], in_=ep[:, n, :])
            for n in range(NCH):
                nc.vector.tensor_mul(
                    esq_t[:, n, :], ep_t[:, n, :ROWS], ep_t[:, n, :ROWS]
                )
            # one PSUM accumulation group of 8:
            #   4x M-block against e (rows 0..16), 4x Z-block against e^2 (row 17)
            for n in range(NCH):
                nc.tensor.matmul(
                    po[:, :], ep_t[:, n, ROWS:ROWS + JL], ep_t[:, n, :ROWS],
                    start=(n == 0), stop=False,
                )
            for n in range(NCH):
                nc.tensor.matmul(
                    po[:, :], ep_t[:, n, ROWS + JL:], esq_t[:, n, :],
                    start=False, stop=(n == NCH - 1),
                )
            nc.vector.tensor_copy(out_t[:, :], po[:, :])
            nc.sync.dma_start(out=out[:, :], in_=out_t[:, :])
    nc.finalize()
    return nc


def _get_nc():
    global _NC_CACHE
    if _NC_CACHE is None:
        _NC_CACHE = build_nc()
    return _NC_CACHE


def _sigmoid64(z):
    return 1.0 / (1.0 + np.exp(-z.astype(np.float64)))


def host_prep(x, emb, gamma, beta, Wq, Wk, Wv, fc_w):
    """Fold weights and shard inputs -> (in_maps, s_vec, c_vec)."""
    f32 = np.float32
    # q at the last position only (full LN of 2 rows, on host)
    er_last = emb[x[:, -1]]                                   # [B,E]
    mu = er_last.mean(-1, keepdims=True)
    var = ((er_last - mu) ** 2).mean(-1, keepdims=True)
    e_last = ((er_last - mu) / np.sqrt(var + EPS)) * gamma + beta
    q = np.einsum("had,bd->bha", Wq, e_last).astype(f32)      # [B,H,A]
    qk = np.einsum("bha,had->bhd", q, Wk).astype(f32)         # [B,H,E]
    g = np.einsum("hv,hvd->hd", fc_w[0].reshape(H, E), Wv).astype(f32)  # [H,E]

    Mb = np.zeros((B, E, JL), f32)
    Mb[:, :, :H] = (qk * gamma / SCALE).transpose(0, 2, 1)
    Mb[:, :, H:2 * H] = (g * gamma).T[None]
    Mb[:, :, 2 * H] = 1.0                                     # sum_e column
    s_vec = Mb[:, :, :2 * H].sum(axis=1)                      # [B,16] col sums
    c_vec = np.concatenate(
        [(qk * beta).sum(-1) / SCALE, np.broadcast_to((g * beta).sum(-1), (B, H))],
        axis=1,
    ).astype(f32)                                             # [B,16]
    Z = np.zeros((E, JL), f32)
    Z[:, 2 * H + 1] = 1.0                                     # sum_e^2 column

    import ml_dtypes
    bf16 = ml_dtypes.bfloat16
    # device layout [partition p, chunk n, col j] for the M and Z blocks
    mm_dev = [
        Mb[b].reshape(NCH, 128, JL).transpose(1, 0, 2).astype(bf16) for b in range(B)
    ]
    z_dev = Z.reshape(NCH, 128, JL).transpose(1, 0, 2).astype(bf16)
    er = emb[x.reshape(-1)]                                   # [B*S, E] gathered rows
    in_maps = []
    for c in range(NCORES):
        blk = er[c * ROWS:(c + 1) * ROWS]                     # [ROWS, E]
        b = (c * ROWS) // S
        ep = np.empty((128, NCH, CW), dtype=bf16)
        ep[:, :, :ROWS] = blk.T.astype(bf16).reshape(NCH, 128, ROWS).transpose(1, 0, 2)
        ep[:, :, ROWS:ROWS + JL] = mm_dev[b]
        ep[:, :, ROWS + JL:] = z_dev
        in_maps.append({"ep": ep})
    return in_maps, s_vec, c_vec


def host_epilogue(outs, s_vec, c_vec, fc_b):
    """outs: [NCORES, J+1, ROWS] device results -> final [B] output."""
    f32 = np.float32
    raw = outs[:, :2 * H, :]                                  # [8,16,512]
    mu = outs[:, 2 * H, :] / E                                # [8,512]
    ex2 = outs[:, 2 * H + 1, :] / E
    var = ex2 - mu * mu
    r = (1.0 / np.sqrt(var + f32(EPS))).astype(f32)

    bidx = (np.arange(NCORES) * ROWS) // S
    cols = (r[:, None, :] * (raw - mu[:, None, :] * s_vec[bidx][:, :, None])
            + c_vec[bidx][:, :, None])                        # [8,16,512]
    cols = cols.reshape(B, 4, 2 * H, ROWS).transpose(0, 2, 1, 3).reshape(B, 2 * H, S)
    scores = cols[:, :H, :]
    p = cols[:, H:, :]

    smax = scores.max(-1, keepdims=True)
    smin = scores.min(-1, keepdims=True)
    norm = (scores - smin) / (smax - smin)
    w = _sigmoid64(norm * f32(20.0) - f32(10.0))
    logit = (w * p.astype(np.float64)).sum((1, 2)) + np.float64(fc_b[0])
    return _sigmoid64(np.asarray(logit)).astype(f32)          # [B]


def kernel(x, emb_table, ln_gamma, ln_beta, W_query, W_key, W_value, fc_w, fc_b):
    f32 = np.float32
    x = np.asarray(x)
    emb = np.asarray(emb_table, dtype=f32)
    gamma = np.asarray(ln_gamma, dtype=f32)
    beta = np.asarray(ln_beta, dtype=f32)
    Wq = np.asarray(W_query, dtype=f32)
    Wk = np.asarray(W_key, dtype=f32)
    Wv = np.asarray(W_value, dtype=f32)
    fcw = np.asarray(fc_w, dtype=f32)
    fcb = np.asarray(fc_b, dtype=f32)

    in_maps, s_vec, c_vec = host_prep(x, emb, gamma, beta, Wq, Wk, Wv, fcw)

    from concourse.bass_utils import run_bass_kernel_spmd
    res = run_bass_kernel_spmd(_get_nc(), in_maps, core_ids=list(range(NCORES)))
    outs = np.stack([r["out"] for r in res.results])          # [8, J+1, ROWS]

    return host_epilogue(outs, s_vec, c_vec, fcb)


# revision 18
# speedup vs baseline: 1.0721x; 1.0106x over previous
"""Trainium2 Bass kernel for nn_EruSelfAttentionModel.

Model (reference):
    e = LayerNorm(emb_table[x]) * gamma + beta                      [B,S,E]
    q,k = per-head projections of e; scores = q @ k^T / sqrt(E)     [B,H,S,S]
    norm = minmax-normalized scores row; sel = max(norm) == 1.0 exactly
    weights = (1-sel)*softmax(norm) + sel*sigmoid(20*norm-10)       -> sigmoid only
    ov = weights @ (W_value @ e); out = sigmoid(fc(concat heads))   [B,S]
    return out[:, -1]                                               [B]

Only the last query position survives, and sel == 1.0 exactly in fp32
(it is (smax-smin)/(smax-smin)), so the softmax branch is multiplied by
exactly zero.  By linearity the value projection and fc fold into a single
vector per head g[h] = fc_w[h*E:(h+1)*E] @ W_value[h], and the score row
folds into qk[b,h] = (W_query[h] @ e_last) @ W_key[h].  Everything the
device must compute per token t is then two groups of dot products
    scores[b,h,t] = LN(e_t) . qk[b,h]/sqrt(E),   p[b,h,t] = LN(e_t) . g[h]
plus per-token LayerNorm statistics.  LN folds through the dot product:
    LN(e) . v = r * (e . (gamma*v) - mu * sum(gamma*v)) + beta . v
so the device computes raw = e_raw^T-block matmuls against a [E, 17]
matrix M = [gamma*qk/SCALE | gamma*g | ones] plus sum(e^2) (via an
on-chip square and a ones-column matmul).  The tiny per-(b,h) epilogue
(min/max over t, sigmoid weights, final weighted sum, output sigmoid)
is O(B*H*S) and runs on host after gathering 36 KB per core.

Sharding: the B*S = 4096 token rows are split into 8 contiguous blocks of
512, one per NeuronCore (cores 0-3 -> batch 0, cores 4-7 -> batch 1).
"""

import math

import numpy as np

B, S, E, A, H = 2, 2048, 512, 64, 8
NCORES = 8
ROWS = B * S // NCORES          # 512 token rows per core
NCH = E // 128                  # 4 contraction chunks of 128
JL = 2 * H + 1                  # 16 data cols + sum_e col
CW = ROWS + JL                  # per-chunk free width: e block | M block
SCALE = math.sqrt(E)
EPS = 1e-5

_NC_CACHE = None


def build_nc():
    """Build the per-core Bass program (same program on all 8 cores)."""
    import concourse.bacc as bacc
    import concourse.tile as tile
    from concourse import mybir

    f32 = mybir.dt.float32
    bf16 = mybir.dt.bfloat16
    nc = bacc.Bacc("TRN2", target_bir_lowering=False, enable_partition_id=False)

    # ep: per-core fused input, [128, NCH, CW] bf16, per chunk n:
    #     ep[p, n, :ROWS]   = e^T (feature d = n*128+p, token t on free)
    #     ep[p, n, ROWS:]   = M[d, :] fold matrix (col 16 = ones -> sum_e row)
    ep = nc.dram_tensor("ep", [128, NCH, CW], bf16, kind="ExternalInput")
    # out rows 0..15: raw dot products e_t . M[:, j]; row 16: sum_d e
    out = nc.dram_tensor("out", [JL, ROWS], f32, kind="ExternalOutput")

    with tile.TileContext(nc) as tc:
        with (
            tc.tile_pool(name="sb", bufs=1) as sb,
            tc.tile_pool(name="ps", bufs=1, space="PSUM") as ps,
        ):
            ep_t = sb.tile([128, NCH, CW], bf16)
            out_t = sb.tile([JL, ROWS], f32)
            po = ps.tile([JL, ROWS], f32)

            # chunked loads, issue split across the two HWDGE queues
            for n in range(NCH):
                eng = nc.sync if n % 2 == 0 else nc.scalar
                eng.dma_start(out=ep_t[:, n, :], in_=ep[:, n, :])
            # one PSUM accumulation group: M-block against e
            for n in range(NCH):
                nc.tensor.matmul(
                    po[:, :], ep_t[:, n, ROWS:], ep_t[:, n, :ROWS],
                    start=(n == 0), stop=(n == NCH - 1),
                )
            # copy + store in halves so the first DMA overlaps the second copy
            HALF = ROWS // 2
            nc.vector.tensor_copy(out_t[:, :HALF], po[:, :HALF])
            nc.sync.dma_start(out=out[:, :HALF], in_=out_t[:, :HALF])
            nc.vector.tensor_copy(out_t[:, HALF:], po[:, HALF:])
            nc.scalar.dma_start(out=out[:, HALF:], in_=out_t[:, HALF:])
    nc.finalize()
    return nc


def _get_nc():
    global _NC_CACHE
    if _NC_CACHE is None:
        _NC_CACHE = build_nc()
    return _NC_CACHE


def _sigmoid64(z):
    return 1.0 / (1.0 + np.exp(-z.astype(np.float64)))


def host_prep(x, emb, gamma, beta, Wq, Wk, Wv, fc_w):
    """Fold weights and shard inputs -> (in_maps, s_vec, c_vec)."""
    f32 = np.float32
    # q at the last position only (full LN of 2 rows, on host)
    er_last = emb[x[:, -1]]                                   # [B,E]
    mu = er_last.mean(-1, keepdims=True)
    var = ((er_last - mu) ** 2).mean(-1, keepdims=True)
    e_last = ((er_last - mu) / np.sqrt(var + EPS)) * gamma + beta
    q = np.einsum("had,bd->bha", Wq, e_last).astype(f32)      # [B,H,A]
    qk = np.einsum("bha,had->bhd", q, Wk).astype(f32)         # [B,H,E]
    g = np.einsum("hv,hvd->hd", fc_w[0].reshape(H, E), Wv).astype(f32)  # [H,E]

    Mb = np.zeros((B, E, JL), f32)
    Mb[:, :, :H] = (qk * gamma / SCALE).transpose(0, 2, 1)
    Mb[:, :, H:2 * H] = (g * gamma).T[None]
    Mb[:, :, 2 * H] = 1.0                                     # sum_e column
    s_vec = Mb[:, :, :2 * H].sum(axis=1)                      # [B,16] col sums
    c_vec = np.concatenate(
        [(qk * beta).sum(-1) / SCALE, np.broadcast_to((g * beta).sum(-1), (B, H))],
        axis=1,
    ).astype(f32)                                             # [B,16]

    import ml_dtypes
    bf16 = ml_dtypes.bfloat16
    # device layout [partition p, chunk n, col j] for the M block
    mm_dev = [
        Mb[b].reshape(NCH, 128, JL).transpose(1, 0, 2).astype(bf16) for b in range(B)
    ]
    er = emb[x.reshape(-1)]                                   # [B*S, E] gathered rows
    in_maps = []
    sumsq = np.empty((NCORES, ROWS), f32)
    for c in range(NCORES):
        blk = er[c * ROWS:(c + 1) * ROWS]                     # [ROWS, E]
        b = (c * ROWS) // S
        ebf = blk.astype(bf16)                                # the values the device sees
        ep = np.empty((128, NCH, CW), dtype=bf16)
        ep[:, :, :ROWS] = ebf.T.reshape(NCH, 128, ROWS).transpose(1, 0, 2)
        ep[:, :, ROWS:] = mm_dev[b]
        in_maps.append({"ep": ep})
        e32 = ebf.astype(f32)
        sumsq[c] = np.einsum("td,td->t", e32, e32)            # sum_d e^2 per token
    return in_maps, s_vec, c_vec, sumsq


def host_epilogue(outs, s_vec, c_vec, sumsq, fc_b):
    """outs: [NCORES, JL, ROWS] device results -> final [B] output."""
    f32 = np.float32
    raw = outs[:, :2 * H, :]                                  # [8,16,512]
    mu = outs[:, 2 * H, :] / E                                # [8,512]
    ex2 = sumsq / E
    var = ex2 - mu * mu
    r = (1.0 / np.sqrt(var + f32(EPS))).astype(f32)

    bidx = (np.arange(NCORES) * ROWS) // S
    cols = (r[:, None, :] * (raw - mu[:, None, :] * s_vec[bidx][:, :, None])
            + c_vec[bidx][:, :, None])                        # [8,16,512]
    cols = cols.reshape(B, 4, 2 * H, ROWS).transpose(0, 2, 1, 3).reshape(B, 2 * H, S)
    scores = cols[:, :H, :]
    p = cols[:, H:, :]

    smax = scores.max(-1, keepdims=True)
    smin = scores.min(-1, keepdims=True)
    norm = (scores - smin) / (smax - smin)
    w = _sigmoid64(norm * f32(20.0) - f32(10.0))
    logit = (w * p.astype(np.float64)).sum((1, 2)) + np.float64(fc_b[0])
    return _sigmoid64(np.asarray(logit)).astype(f32)          # [B]


def kernel(x, emb_table, ln_gamma, ln_beta, W_query, W_key, W_value, fc_w, fc_b):
    f32 = np.float32
    x = np.asarray(x)
    emb = np.asarray(emb_table, dtype=f32)
    gamma = np.asarray(ln_gamma, dtype=f32)
    beta = np.asarray(ln_beta, dtype=f32)
    Wq = np.asarray(W_query, dtype=f32)
    Wk = np.asarray(W_key, dtype=f32)
    Wv = np.asarray(W_value, dtype=f32)
    fcw = np.asarray(fc_w, dtype=f32)
    fcb = np.asarray(fc_b, dtype=f32)

    in_maps, s_vec, c_vec, sumsq = host_prep(x, emb, gamma, beta, Wq, Wk, Wv, fcw)

    from concourse.bass_utils import run_bass_kernel_spmd
    res = run_bass_kernel_spmd(_get_nc(), in_maps, core_ids=list(range(NCORES)))
    outs = np.stack([r["out"] for r in res.results])          # [8, JL, ROWS]

    return host_epilogue(outs, s_vec, c_vec, sumsq, fcb)
